# revision 26
# baseline (speedup 1.0000x reference)
"""Graph-matching network (2x GAT + mutual cosine attention + matching + MLP)
on 8 Trainium2 NeuronCores, SPMD with node rows sharded 8 ways.

GAT edges are sharded by destination block; softmax uses a global-max shift
(mathematically exact: any per-dst-constant shift cancels); aggregation is
one-hot(exp-weight) matmuls over dma_gather'ed neighbor rows. The NxN mutual
attention is computed transposed (cosT[j,i]) from pre-norm-scaled operands,
with the column sum folded in as one extra rhs column, AllReduced, and folded
into the second matmul's rhs. Everything downstream is row-local with small
AllReduces for the readouts.
"""
import sys

for _p in ("/opt/trn_rl_repo", "/root/.axon_site/_ro/trn_rl_repo"):
    if _p not in sys.path:
        sys.path.insert(0, _p)

import os
import numpy as np
import ml_dtypes

STAGE = int(os.environ.get("KSTAGE", "9"))
NODVE = os.environ.get("K_NODVE", "") == "1"
NOMM = os.environ.get("K_NOMM", "") == "1"
NOGA = os.environ.get("K_NOGA", "") == "1"


def _dbg_out(nc, z_out, ro, tagv):
    pass

import concourse.bass as bass
import concourse.bacc as bacc
import concourse.mybir as mybir
import concourse.tile as tile
from concourse.bass_utils import run_bass_kernel_spmd
from concourse.masks import make_identity

P = 128
NC = 8
N = 8192
NL = N // NC          # 1024 rows per core
NT = NL // P          # 8 node tiles per core
F = 128
H = 512
D = 256
DT = D // P           # 2
MS = 256
MT = MS // P          # 2
JT = N // P           # 64
f32 = mybir.dt.float32
bf16 = mybir.dt.bfloat16
i16 = mybir.dt.int16
bf = ml_dtypes.bfloat16
AF = mybir.ActivationFunctionType
OP = mybir.AluOpType
AX = mybir.AxisListType
RG = [list(range(NC))]

W1EXT = 640   # h(0:512) hs(512) one(513) hd(514) pad -> 640 (1280B rows)
W2EXT = 384   # h(0:256) hs(256) one(257) hd(258) pad -> 384 (768B rows)
SB = 16       # gather sub-batch size in chunks of 128 edges

_CACHE = {}


# ----------------------------------------------------------------- host prep
def _prep_edges(ei):
    src = np.concatenate([ei[0], ei[1], np.arange(N)]).astype(np.int64)
    dst = np.concatenate([ei[1], ei[0], np.arange(N)]).astype(np.int64)
    order = np.argsort(dst, kind="stable")
    return src[order], dst[order]


def _shard_edges(srcs, dsts, EB):
    out = []
    for c in range(NC):
        s_arr = np.zeros(8 * EB, np.int64)
        d_arr = np.zeros(8 * EB, np.int64)
        rel = np.full(8 * EB, -1.0, np.float32)
        for b in range(8):
            base = c * NL + b * P
            l2 = np.searchsorted(dsts, base)
            h2 = np.searchsorted(dsts, base + P)
            n = h2 - l2
            assert n <= EB
            s_arr[b * EB: b * EB + n] = srcs[l2:h2]
            d_arr[b * EB: b * EB + n] = dsts[l2:h2]
            rel[b * EB: b * EB + n] = (dsts[l2:h2] - base).astype(np.float32)
        out.append((s_arr, d_arr, rel))
    return out


def _wrap_idx(idx):
    e = idx.shape[0]
    w = idx.reshape(e // 16, 16).T.astype(np.int16)
    return np.tile(w, (8, 1)).copy()


def _rel_layout(rel):
    e = rel.shape[0]
    return rel.reshape(e // P, P).T.astype(bf).copy()


def _host_prep(inputs):
    inp = {k: np.asarray(v) for k, v in inputs.items()}
    es = _prep_edges(inp["edge_index_s"].astype(np.int64))
    et = _prep_edges(inp["edge_index_t"].astype(np.int64))
    maxcnt = 0
    for srcs, dsts in (es, et):
        cnt = np.bincount(dsts // P, minlength=N // P)
        maxcnt = max(maxcnt, int(cnt.max()))
    EB = -(-maxcnt // P) * P
    shards_s = _shard_edges(*es, EB)
    shards_t = _shard_edges(*et, EB)

    W1e = np.concatenate(
        [inp["W_g1"], (inp["W_g1"] @ inp["a1_src"])[:, None],
         (inp["W_g1"] @ inp["a1_dst"])[:, None]], 1).astype(np.float32)
    W2e = np.concatenate(
        [inp["W_g2"], (inp["W_g2"] @ inp["a2_src"])[:, None],
         (inp["W_g2"] @ inp["a2_dst"])[:, None]], 1).astype(np.float32)
    W2m = (inp["W_match"] * inp["W_match"]).astype(np.float32)

    in_maps = []
    for c in range(NC):
        m = {}
        rows = slice(c * NL, (c + 1) * NL)
        m["xT_s"] = np.ascontiguousarray(inp["x_s"][rows].T.astype(np.float32))
        m["xT_t"] = np.ascontiguousarray(inp["x_t"][rows].T.astype(np.float32))
        for g, shards in (("s", shards_s), ("t", shards_t)):
            s_arr, d_arr, rel = shards[c]
            m[f"srcw_{g}"] = _wrap_idx(s_arr)
            m[f"dstw_{g}"] = _wrap_idx(d_arr)
            m[f"rel_{g}"] = _rel_layout(rel)
        m["W1e"] = W1e
        m["W2e"] = W2e
        m["W_read"] = inp["W_read"].astype(np.float32)
        m["W2m"] = W2m
        for i in (1, 2, 3, 4):
            m[f"mlp_w{i}"] = inp[f"mlp_w{i}"].astype(np.float32)
        m["mlp_b1"] = np.ascontiguousarray(
            inp["mlp_b1"].astype(np.float32).reshape(4, P).T)
        m["mlp_b2"] = np.ascontiguousarray(
            inp["mlp_b2"].astype(np.float32).reshape(2, P).T)
        m["mlp_b3"] = np.ascontiguousarray(
            inp["mlp_b3"].astype(np.float32).reshape(1, P).T)
        m["mlp_b4"] = inp["mlp_b4"].astype(np.float32).reshape(1, 1).copy()
        in_maps.append(m)
    return in_maps, EB


# ------------------------------------------------------------------- builder
class G:
    pass


def build(EB):
    CB = EB // P
    subs = [(i, min(SB, CB - i)) for i in range(0, CB, SB)]

    nc = bacc.Bacc("TRN2", target_bir_lowering=False, debug=False,
                   num_devices=NC)
    ins = {}
    for c0 in ["xT_s", "xT_t"]:
        ins[c0] = nc.dram_tensor(c0, [F, NL], f32, kind="ExternalInput")
    for g in "st":
        ins[f"srcw_{g}"] = nc.dram_tensor(f"srcw_{g}", [P, 8 * EB // 16], i16,
                                          kind="ExternalInput")
        ins[f"dstw_{g}"] = nc.dram_tensor(f"dstw_{g}", [P, 8 * EB // 16], i16,
                                          kind="ExternalInput")
        ins[f"rel_{g}"] = nc.dram_tensor(f"rel_{g}", [P, 8 * EB // P], bf16,
                                         kind="ExternalInput")
    for nm, sh in [("W1e", [F, H + 2]), ("W2e", [H, D + 2]),
                   ("W_read", [D, D]), ("W2m", [D, MS]),
                   ("mlp_w1", [8 * MS, 2 * MS]), ("mlp_w2", [2 * MS, MS]),
                   ("mlp_w3", [MS, MS // 2]), ("mlp_w4", [MS // 2, 1]),
                   ("mlp_b1", [P, 4]), ("mlp_b2", [P, 2]), ("mlp_b3", [P, 1]),
                   ("mlp_b4", [1, 1])]:
        ins[nm] = nc.dram_tensor(nm, sh, f32, kind="ExternalInput")
    z_out = nc.dram_tensor("z", [1, 1], f32, kind="ExternalOutput")

    with tile.TileContext(nc) as tc:
        _build_tc(tc, nc, ins, z_out, EB, CB, subs)
    nc.compile()
    return nc


def _build_tc(tc, nc, ins, z_out, EB, CB, subs):
    from contextlib import ExitStack
    ctx = ExitStack()
    const = ctx.enter_context(tc.tile_pool(name="const", bufs=1))
    dram = ctx.enter_context(tc.tile_pool(name="dram", bufs=1, space="DRAM"))
    sb = ctx.enter_context(tc.tile_pool(name="persist", bufs=1))
    ps = ctx.enter_context(tc.tile_pool(name="ps", bufs=2, space="PSUM"))

    # psum tags (8 banks total):
    #  "big"  [P,1024] (2 banks, bufs=2 -> 4)  mutual pass1
    #  "main" [P,512]  (1 bank,  bufs=2 -> 2)  everything medium
    #  "col"  [P,128]  (1 bank,  bufs=2 -> 2)  columns + transposes
    def ps_big():
        return ps.tile([P, NL], f32, tag="big", name="psbig")

    def ps_main(w=512):
        t = ps.tile([P, 512], f32, tag="main", name="psmain")
        return t[:, 0:w] if w != 512 else t

    def ps_col(w=1):
        t = ps.tile([P, P], f32, tag="col", name="pscol")
        return t[:, 0:w] if w != P else t

    ident = const.tile([P, P], f32)
    make_identity(nc, ident[:])
    identb = const.tile([P, P], bf16)
    nc.vector.tensor_copy(identb[:], ident[:])
    iota_rep = const.tile([P, P], bf16)
    nc.gpsimd.iota(iota_rep[:], pattern=[[1, P]], base=0, channel_multiplier=0,
                   allow_small_or_imprecise_dtypes=True)
    onescol = const.tile([P, 1], bf16)
    nc.gpsimd.memset(onescol[:], 1.0)
    invN = const.tile([P, 1], bf16)
    nc.gpsimd.memset(invN[:], 1.0 / N)

    def pe_transpose(src_ap, dst_ap, fp32=False):
        # plain matmul vs identity: out = src.T (avoids PE transpose-mode,
        # which crashes when the scheduler interleaves it into matmul groups)
        pt = ps_col(P)
        nc.tensor.matmul(out=pt, lhsT=src_ap,
                         rhs=(ident[:] if fp32 else identb[:]),
                         start=True, stop=True)
        nc.vector.tensor_copy(dst_ap, pt)

    def allreduce(vals_ap, shape, op=OP.add):
        bi = dram.tile(shape, f32, tag="arb_i")
        bo = dram.tile(shape, f32, tag="arb_o")
        nc.sync.dma_start(bi[:], vals_ap)
        nc.gpsimd.collective_compute("AllReduce", op, replica_groups=RG,
                                     ins=[bi.opt()], outs=[bo.opt()])
        return bo

    def ar_to_rep(vals_ap, width, pool, op=OP.add, tag="rep"):
        """AllReduce a [1, width] row; return [P, width] replicated f32."""
        bo = allreduce(vals_ap, [1, width], op=op)
        row = pool.tile([1, width], f32, tag=tag + "_row")
        nc.sync.dma_start(row[:], bo[:])
        rep = pool.tile([P, width], f32, tag=tag)
        nc.gpsimd.partition_broadcast(rep[:], row[:], channels=P)
        return rep

    def partreduce(src_ap, k, op, pool):
        """Reduce [P, k] across partitions -> [1, k] row (k <= 128)."""
        pad = pool.tile([P, P], f32, tag="pr_pad")
        nc.gpsimd.memset(pad[:], -1e30 if op == OP.max else 0.0)
        nc.vector.tensor_copy(pad[:, 0:k], src_ap)
        tp = ps_col(P)
        nc.tensor.matmul(out=tp, lhsT=pad[:], rhs=ident[:], start=True,
                         stop=True)
        tps = pool.tile([P, P], f32, tag="pr_t")
        nc.vector.tensor_copy(tps[:], tp)
        red = pool.tile([P, 1], f32, tag="pr_r")
        if op == OP.max:
            nc.vector.reduce_max(red[:], tps[:], AX.X)
        else:
            nc.vector.reduce_sum(red[:], tps[:], AX.X)
        pad2 = pool.tile([P, P], f32, tag="pr_pad2")
        nc.gpsimd.memset(pad2[:], 0.0)
        nc.vector.tensor_copy(pad2[:, 0:1], red[:])
        tp2 = ps_col(P)
        nc.tensor.matmul(out=tp2, lhsT=pad2[:], rhs=ident[:], start=True,
                         stop=True)
        out = pool.tile([1, k], f32, tag="pr_row")
        nc.vector.tensor_copy(out[:], tp2[0:1, 0:k])
        return out

    gs = {"s": G(), "t": G()}

    # ================= phase 0+1: stats, h_pre, h_ext1 =================
    ph1 = ExitStack()
    p1 = ph1.enter_context(tc.tile_pool(name="p1", bufs=1))
    stat = p1.tile([P, 4], f32)
    for gi, g in enumerate("st"):
        xt = p1.tile([F, NL], f32, tag=f"xT_{g}")
        nc.sync.dma_start(xt[:], ins[f"xT_{g}"][:])
        gs[g].xT = xt
        nc.vector.reduce_sum(stat[:, 2 * gi:2 * gi + 1], xt[:], AX.X)
        sq = p1.tile([F, NL], f32, tag="sqtmp")
        nc.scalar.activation(sq[:], xt[:], AF.Square)
        nc.vector.reduce_sum(stat[:, 2 * gi + 1:2 * gi + 2], sq[:], AX.X)
    srow = partreduce(stat[:], 4, OP.add, p1)
    srep = ar_to_rep(srow[:], 4, p1, tag="stats")
    NF = float(N * F)
    musd = p1.tile([P, 4], f32)   # mu_s rstd_s mu_t rstd_t
    tmp4 = p1.tile([P, 4], f32)
    for gi in range(2):
        s1 = srep[:, 2 * gi:2 * gi + 1]
        s2 = srep[:, 2 * gi + 1:2 * gi + 2]
        nc.vector.tensor_scalar(out=musd[:, 2 * gi:2 * gi + 1], in0=s1,
                                scalar1=1.0 / NF, scalar2=None, op0=OP.mult)
        nc.scalar.activation(tmp4[:, 0:1], musd[:, 2 * gi:2 * gi + 1], AF.Square)
        nc.vector.tensor_scalar(out=tmp4[:, 1:2], in0=tmp4[:, 0:1], scalar1=NF,
                                scalar2=None, op0=OP.mult)
        nc.vector.tensor_tensor(out=tmp4[:, 2:3], in0=s2, in1=tmp4[:, 1:2],
                                op=OP.subtract)
        nc.vector.tensor_scalar(out=tmp4[:, 3:4], in0=tmp4[:, 2:3],
                                scalar1=1.0 / (NF - 1.0), scalar2=None,
                                op0=OP.mult)
        nc.scalar.activation(tmp4[:, 0:1], tmp4[:, 3:4], AF.Sqrt)
        nc.vector.reciprocal(musd[:, 2 * gi + 1:2 * gi + 2], tmp4[:, 0:1])

    w1b = p1.tile([F, H + 2], bf16)
    nc.gpsimd.dma_start(w1b[:], ins["W1e"][:])

    for gi, g in enumerate("st"):
        st = gs[g]
        xt = st.xT
        nc.vector.tensor_scalar(out=xt[:], in0=xt[:],
                                scalar1=musd[:, 2 * gi:2 * gi + 1],
                                scalar2=musd[:, 2 * gi + 1:2 * gi + 2],
                                op0=OP.subtract, op1=OP.mult)
        xb = p1.tile([F, NL], bf16, tag=f"xb_{g}")
        nc.vector.tensor_copy(xb[:], xt[:])
        hext = p1.tile([P, NT, W1EXT], bf16, tag=f"hext1_{g}")
        st.hext1 = hext
        nc.gpsimd.memset(hext[:], 0.0)
        nc.gpsimd.memset(hext[:, :, H + 1:H + 2], 1.0)
        hmax = p1.tile([P, 2], f32, tag=f"hmax_{g}")
        nc.gpsimd.memset(hmax[:], -1e30)
        for nt in range(NT):
            ph = ps_main(H)
            pv = ps_col(2)
            nc.tensor.matmul(out=ph, lhsT=xb[:, nt * P:(nt + 1) * P],
                             rhs=w1b[:, 0:H], start=True, stop=True)
            nc.tensor.matmul(out=pv, lhsT=xb[:, nt * P:(nt + 1) * P],
                             rhs=w1b[:, H:H + 2], start=True, stop=True)
            nc.vector.tensor_copy(hext[:, nt, 0:H], ph)
            nc.vector.tensor_copy(hext[:, nt, H:H + 1], pv[:, 0:1])
            nc.vector.tensor_copy(hext[:, nt, H + 2:H + 3], pv[:, 1:2])
            nc.vector.tensor_tensor(out=hmax[:], in0=hmax[:], in1=pv, op=OP.max)
        st.hmax1 = hmax

    hm2 = p1.tile([P, 4], f32)
    nc.vector.tensor_copy(hm2[:, 0:2], gs["s"].hmax1[:])
    nc.vector.tensor_copy(hm2[:, 2:4], gs["t"].hmax1[:])
    mrow = partreduce(hm2[:], 4, OP.max, p1)
    mrep = ar_to_rep(mrow[:], 4, p1, op=OP.max, tag="smax1")

    def shift_cols(rep, pool, tag):
        out = pool.tile([P, 2], f32, tag=tag)
        a = pool.tile([P, 2], f32, tag=tag + "_t")
        for gi in range(2):
            nc.vector.tensor_tensor(out=a[:, 0:1], in0=rep[:, 2 * gi:2 * gi + 1],
                                    in1=rep[:, 2 * gi + 1:2 * gi + 2], op=OP.add)
            nc.vector.tensor_scalar(out=a[:, 1:2], in0=a[:, 0:1], scalar1=0.2,
                                    scalar2=None, op0=OP.mult)
            nc.vector.tensor_tensor(out=a[:, 0:1], in0=a[:, 0:1], in1=a[:, 1:2],
                                    op=OP.max)
            nc.vector.tensor_scalar(out=out[:, gi:gi + 1], in0=a[:, 0:1],
                                    scalar1=-1.0, scalar2=None, op0=OP.mult)
        return out

    negs1 = sb.tile([P, 2], f32, tag="negs1")
    nc.vector.tensor_copy(negs1[:], shift_cols(mrep, p1, "sh1")[:])

    for g in "st":
        st = gs[g]
        shard = dram.tile([NL, W1EXT], bf16, tag=f"he1s_{g}")
        full = dram.tile([N, W1EXT], bf16, tag=f"he1f_{g}")
        nc.sync.dma_start(shard.rearrange("(t p) w -> p t w", p=P), st.hext1[:])
        nc.gpsimd.collective_compute("AllGather", OP.bypass, replica_groups=RG,
                                     ins=[shard.opt()], outs=[full.opt()])
        st.hext1_full = full
    ph1.close()
    if STAGE == 15:
        # single gather from the AG output
        dbg = sb.tile([P, 1, W1EXT], bf16, tag="dbg15")
        idxd = sb.tile([P, 8], i16, tag="idx15")
        nc.sync.dma_start(idxd[:], ins["srcw_s"][:, 0:8])
        nc.gpsimd.dma_gather(dbg[:], gs["s"].hext1_full[:], idxd[:], P, P,
                             W1EXT)
        zf0 = sb.tile([1, 1], f32, tag="zf0")
        nc.vector.tensor_copy(zf0[:], dbg[0:1, 0, 0:1])
        nc.sync.dma_start(z_out[:], zf0[:])
        ctx.close()
        return
    if STAGE == 16:
        # single gather from the shard (non-AG dram tile): use first shard
        dbg = sb.tile([P, 1, W1EXT], bf16, tag="dbg15")
        idxd = sb.tile([P, 8], i16, tag="idx15")
        nc.sync.dma_start(idxd[:], ins["srcw_s"][:, 0:8])
        sh = dram.tile([NL, W1EXT], bf16, tag="he1s_s")
        nc.gpsimd.dma_gather(dbg[:], sh[:], idxd[:], P, P, W1EXT)
        zf0 = sb.tile([1, 1], f32, tag="zf0")
        nc.vector.tensor_copy(zf0[:], dbg[0:1, 0, 0:1])
        nc.sync.dma_start(z_out[:], zf0[:])
        ctx.close()
        return
    if STAGE == 17:
        dbg = sb.tile([P, SB, W1EXT], bf16, tag="dbg17")
        dbg2 = sb.tile([P, SB, P], bf16, tag="dbg17b")
        idxd = sb.tile([P, SB * 8], i16, tag="idx17")
        nc.sync.dma_start(idxd[:], ins["srcw_s"][:, 0:SB * 8])
        nc.gpsimd.dma_gather(dbg[:], gs["s"].hext1_full[:], idxd[:], SB * P,
                             SB * P, W1EXT)
        idxd2 = sb.tile([P, SB * 8], i16, tag="idx17c")
        nc.sync.dma_start(idxd2[:], ins["dstw_s"][:, 0:SB * 8])
        nc.gpsimd.dma_gather(dbg2[:], gs["s"].hext1_full[:, H:H + P], idxd2[:],
                             SB * P, SB * P, P, elem_step=W1EXT)
        zf0 = sb.tile([1, 1], f32, tag="zf0")
        nc.vector.tensor_copy(zf0[:], dbg2[0:1, 0, 2:3])
        nc.sync.dma_start(z_out[:], zf0[:])
        ctx.close()
        return
    if STAGE == 18:
        dbg = sb.tile([P, SB, W1EXT], bf16, tag="dbg17")
        idxd = sb.tile([P, SB * 8], i16, tag="idx17")
        nc.sync.dma_start(idxd[:], ins["srcw_s"][:, 0:SB * 8])
        nc.gpsimd.dma_gather(dbg[:], gs["s"].hext1_full[:], idxd[:], SB * P,
                             SB * P, W1EXT,
                             single_packet=(os.environ.get("K_SP", "1") == "1"))
        zf0 = sb.tile([1, 1], f32, tag="zf0")
        nc.vector.tensor_copy(zf0[:], dbg[0:1, 0, 0:1])
        nc.sync.dma_start(z_out[:], zf0[:])
        ctx.close()
        return
    if STAGE == 19:
        dbg2 = sb.tile([P, 1, P], bf16, tag="dbg17b")
        idxd2 = sb.tile([P, 8], i16, tag="idx17c")
        nc.sync.dma_start(idxd2[:], ins["dstw_s"][:, 0:8])
        nc.gpsimd.dma_gather(dbg2[:], gs["s"].hext1_full[:, H:H + P], idxd2[:],
                             P, P, P, elem_step=W1EXT)
        zf0 = sb.tile([1, 1], f32, tag="zf0")
        nc.vector.tensor_copy(zf0[:], dbg2[0:1, 0, 2:3])
        nc.sync.dma_start(z_out[:], zf0[:])
        ctx.close()
        return
    if STAGE <= 1:
        zf0 = sb.tile([1, 1], f32, tag="zf0")
        nc.gpsimd.memset(zf0[:], 1.0)
        nc.sync.dma_start(z_out[:], zf0[:])
        ctx.close()
        return

    # ================= GAT layers =================
    gph = ExitStack()
    gat = gph.enter_context(tc.tile_pool(name="gat", bufs=2))
    gat1 = gph.enter_context(tc.tile_pool(name="gat1", bufs=1))
    w2b = gat1.tile([P, H // P, D + 2], bf16, tag="w2b")
    nc.gpsimd.dma_start(w2b[:], ins["W2e"][:].rearrange("(a p) w -> p a w", p=P))

    def gat_layer(g, lay):
        st = gs[g]
        cmp_z = gat1.tile([P, SB, P], bf16, tag="cmpz", name="cmpz") if NODVE else None
        gi = 0 if g == "s" else 1
        WEXT = W1EXT if lay == 1 else W2EXT
        HD = H if lay == 1 else D
        hfull = st.hext1_full if lay == 1 else st.hext2_full
        negs = negs1 if lay == 1 else negs2
        srcw = gat1.tile([P, 8 * EB // 16], i16, tag="srcw")
        dstw = gat1.tile([P, 8 * EB // 16], i16, tag="dstw")
        relx = gat1.tile([P, 8 * EB // P], bf16, tag="relx")
        nc.sync.dma_start(srcw[:], ins[f"srcw_{g}"][:])
        nc.sync.dma_start(dstw[:], ins[f"dstw_{g}"][:])
        nc.sync.dma_start(relx[:], ins[f"rel_{g}"][:])
        if lay == 1:
            st.hext2 = gat1.tile([P, NT, W2EXT], bf16, tag=f"hext2_{g}")
            nc.gpsimd.memset(st.hext2[:], 0.0)
            nc.gpsimd.memset(st.hext2[:, :, D + 1:D + 2], 1.0)
            st.hmax2 = gat1.tile([P, 2], f32, tag=f"hmax2_{g}")
            nc.gpsimd.memset(st.hmax2[:], -1e30)
        else:
            st.h_rows = gat1.tile([P, NT, D], f32, tag=f"hrows_{g}")
        for b in range(8):
            U = ps_main(HD)
            dn = ps_col(1)
            for (s0, ns) in subs:
                nidx = ns * P
                V = gat.tile([P, SB, WEXT], bf16, tag="Vg")
                Vhd = gat.tile([P, SB, P], bf16, tag="Vhd")
                i0 = (b * CB + s0) * 8
                if not NOGA:
                    nc.gpsimd.dma_gather(V[:, 0:ns, :], hfull[:],
                                         srcw[:, i0:i0 + nidx // 16], nidx,
                                         nidx, WEXT, single_packet=False)
                    nc.gpsimd.dma_gather(Vhd[:, 0:ns, :], hfull[:, HD:HD + P],
                                         dstw[:, i0:i0 + nidx // 16], nidx,
                                         nidx, P, elem_step=WEXT,
                                         single_packet=False)
                else:
                    nc.gpsimd.memset(V[:, 0:ns, :], 0.001)
                    nc.gpsimd.memset(Vhd[:, 0:ns, :], 0.001)
                if NODVE:
                    nc.gpsimd.memset(cmp_z[:], 0.0)
                lv = gat.tile([P, SB], f32, tag="lv")
                if NODVE:
                    cmp = cmp_z
                t2 = gat.tile([P, SB], f32, tag="lv2")
                if NODVE:
                    pass
                else:
                    nc.vector.tensor_copy(lv[:, 0:ns], V[:, 0:ns, HD])
                if not NODVE:
                    nc.vector.tensor_copy(t2[:, 0:ns], Vhd[:, 0:ns, 2])
                    nc.vector.tensor_tensor(out=lv[:, 0:ns], in0=lv[:, 0:ns],
                                            in1=t2[:, 0:ns], op=OP.add)
                    nc.vector.tensor_scalar(out=t2[:, 0:ns], in0=lv[:, 0:ns],
                                            scalar1=0.2, scalar2=None,
                                            op0=OP.mult)
                    nc.vector.tensor_tensor(out=lv[:, 0:ns], in0=lv[:, 0:ns],
                                            in1=t2[:, 0:ns], op=OP.max)
                    nc.scalar.activation(lv[:, 0:ns], lv[:, 0:ns], AF.Exp,
                                         bias=negs[:, gi:gi + 1], scale=1.0)
                    exb = gat.tile([P, SB], bf16, tag="exb")
                    nc.vector.tensor_copy(exb[:, 0:ns], lv[:, 0:ns])
                    cmp = gat.tile([P, SB, P], bf16, tag="cmp")
                    nc.vector.tensor_tensor(
                        out=cmp[:, 0:ns, :],
                        in0=relx[:, b * CB + s0:b * CB + s0 + ns, None]
                            .broadcast_to((P, ns, P)),
                        in1=iota_rep[:, None, :].broadcast_to((P, ns, P)),
                        op=OP.is_equal)
                    nc.vector.tensor_tensor(
                        out=cmp[:, 0:ns, :], in0=cmp[:, 0:ns, :],
                        in1=exb[:, 0:ns, None].broadcast_to((P, ns, P)),
                        op=OP.mult)
                if NOMM:
                    continue
                for j in range(ns):
                    first = (s0 == 0 and j == 0)
                    last = (s0 + ns == CB and j == ns - 1)
                    nc.tensor.matmul(out=U, lhsT=cmp[:, j, :],
                                     rhs=V[:, j, 0:HD],
                                     start=first, stop=last)
                    nc.tensor.matmul(out=dn, lhsT=cmp[:, j, :],
                                     rhs=V[:, j, HD + 1:HD + 2],
                                     start=first, stop=last)
            if NOMM:
                if lay == 1:
                    nc.gpsimd.memset(st.hext2[:, b, :], 0.001)
                    nc.gpsimd.memset(st.hext2[:, b, D + 1:D + 2], 1.0)
                else:
                    nc.gpsimd.memset(st.h_rows[:, b, :], 0.001)
                continue
            rec = gat.tile([P, 1], f32, tag="recip")
            nc.vector.tensor_scalar(out=rec[:], in0=dn, scalar1=1e-16,
                                    scalar2=None, op0=OP.add)
            nc.vector.reciprocal(rec[:], rec[:])
            hb = gat.tile([P, HD], f32, tag="hblk")
            nc.scalar.activation(hb[:], U, AF.Copy, scale=rec[:])
            if lay == 1:
                t0 = gat.tile([P, HD], f32, tag="elu0")
                nc.vector.tensor_scalar(out=t0[:], in0=hb[:], scalar1=0.0,
                                        scalar2=None, op0=OP.min)
                nc.scalar.activation(t0[:], t0[:], AF.Exp)
                nc.vector.tensor_scalar(out=hb[:], in0=hb[:], scalar1=0.0,
                                        scalar2=None, op0=OP.max)
                nc.vector.tensor_tensor(out=hb[:], in0=hb[:], in1=t0[:],
                                        op=OP.add)
                nc.vector.tensor_scalar(out=hb[:], in0=hb[:], scalar1=1.0,
                                        scalar2=None, op0=OP.subtract)
                hbb = gat.tile([P, HD], bf16, tag="hblkb")
                nc.vector.tensor_copy(hbb[:], hb[:])
                p2 = ps_main(D)
                pv2 = ps_col(2)
                for kt in range(H // P):
                    tT = gat.tile([P, P], bf16, tag="hbT")
                    pe_transpose(hbb[:, kt * P:(kt + 1) * P], tT[:])
                    nc.tensor.matmul(out=p2, lhsT=tT[:], rhs=w2b[:, kt, 0:D],
                                     start=(kt == 0), stop=(kt == H // P - 1))
                    nc.tensor.matmul(out=pv2, lhsT=tT[:],
                                     rhs=w2b[:, kt, D:D + 2],
                                     start=(kt == 0), stop=(kt == H // P - 1))
                nc.vector.tensor_copy(st.hext2[:, b, 0:D], p2)
                nc.vector.tensor_copy(st.hext2[:, b, D:D + 1], pv2[:, 0:1])
                nc.vector.tensor_copy(st.hext2[:, b, D + 2:D + 3], pv2[:, 1:2])
                nc.vector.tensor_tensor(out=st.hmax2[:], in0=st.hmax2[:],
                                        in1=pv2, op=OP.max)
            else:
                nc.vector.tensor_copy(st.h_rows[:, b, :], hb[:])

    for g in "st":
        gat_layer(g, 1)
    if STAGE <= 2:
        zf0 = sb.tile([1, 1], f32, tag="zf0")
        nc.gpsimd.memset(zf0[:], 1.0)
        nc.sync.dma_start(z_out[:], zf0[:])
        gph.close()
        ctx.close()
        return
    hm2b = gat1.tile([P, 4], f32, tag="hm2b")
    nc.vector.tensor_copy(hm2b[:, 0:2], gs["s"].hmax2[:])
    nc.vector.tensor_copy(hm2b[:, 2:4], gs["t"].hmax2[:])
    mrow2 = partreduce(hm2b[:], 4, OP.max, gat1)
    m2rep = ar_to_rep(mrow2[:], 4, gat1, op=OP.max, tag="smax2")
    negs2 = sb.tile([P, 2], f32, tag="negs2")
    nc.vector.tensor_copy(negs2[:], shift_cols(m2rep, gat1, "sh2")[:])
    for g in "st":
        st = gs[g]
        shard = dram.tile([NL, W2EXT], bf16, tag=f"he2s_{g}")
        full = dram.tile([N, W2EXT], bf16, tag=f"he2f_{g}")
        nc.sync.dma_start(shard.rearrange("(t p) w -> p t w", p=P),
                          st.hext2[:])
        nc.gpsimd.collective_compute("AllGather", OP.bypass, replica_groups=RG,
                                     ins=[shard.opt()], outs=[full.opt()])
        st.hext2_full = full
    for g in "st":
        gat_layer(g, 2)

    # ================= norms, transposes, AG of h =================
    for g in "st":
        st = gs[g]
        nsq = gat1.tile([P, NT], f32, tag="nsq")
        for t in range(NT):
            sq = gat.tile([P, D], f32, tag="sqr")
            nc.vector.tensor_tensor(out=sq[:], in0=st.h_rows[:, t, :],
                                    in1=st.h_rows[:, t, :], op=OP.mult)
            nc.vector.reduce_sum(nsq[:, t:t + 1], sq[:], AX.X)
        nrm = sb.tile([P, NT], f32, tag=f"nrm_{g}")
        nc.scalar.activation(nrm[:], nsq[:], AF.Sqrt)
        nc.vector.tensor_scalar(out=nrm[:], in0=nrm[:], scalar1=1e-6,
                                scalar2=None, op0=OP.max)
        st.nrm = nrm
        rn = sb.tile([P, NT], f32, tag=f"rn_{g}")
        nc.vector.reciprocal(rn[:], nrm[:])
        st.rn = rn
        st.h_rows_b = sb.tile([P, NT, D], bf16, tag=f"hrb_{g}")
        nc.vector.tensor_copy(st.h_rows_b[:], st.h_rows[:])
        hsr = gat1.tile([P, NT, D], bf16, tag="hsr")
        for t in range(NT):
            nc.vector.tensor_scalar(out=hsr[:, t, :], in0=st.h_rows[:, t, :],
                                    scalar1=rn[:, t:t + 1], scalar2=None,
                                    op0=OP.mult)
        st.hTs = sb.tile([P, DT, NL], bf16, tag=f"hTs_{g}")
        st.hT = sb.tile([P, DT, NL], bf16, tag=f"hT_{g}")
        for t in range(NT):
            for ft in range(DT):
                pe_transpose(hsr[:, t, ft * P:(ft + 1) * P],
                             st.hTs[:, ft, t * P:(t + 1) * P])
                pe_transpose(st.h_rows_b[:, t, ft * P:(ft + 1) * P],
                             st.hT[:, ft, t * P:(t + 1) * P])
        shT = dram.tile([D, NL], bf16, tag=f"shT_{g}")
        fullT = dram.tile([NC, D, NL], bf16, tag=f"fT_{g}")
        nc.sync.dma_start(shT.rearrange("(a p) w -> p a w", p=P), st.hTs[:])
        shR = dram.tile([NL, D], bf16, tag=f"shR_{g}")
        fullR = dram.tile([N, D], bf16, tag=f"fR_{g}")
        nc.sync.dma_start(shR.rearrange("(t p) w -> p t w", p=P),
                          st.h_rows_b[:])
        nc.gpsimd.collective_compute("AllGather", OP.bypass, replica_groups=RG,
                                     ins=[shT.opt()], outs=[fullT.opt()])
        nc.gpsimd.collective_compute("AllGather", OP.bypass, replica_groups=RG,
                                     ins=[shR.opt()], outs=[fullR.opt()])
        st.hTs_full = fullT
        st.hR_full = fullR
    gph.close()
    if STAGE <= 3:
        zf0 = sb.tile([1, 1], f32, tag="zf0")
        nc.vector.tensor_copy(zf0[:], gs["s"].h_rows_b[0:1, 0, 0:1])
        nc.sync.dma_start(z_out[:], zf0[:])
        ctx.close()
        return

    # ================= readout helpers =================
    W_read = sb.tile([P, DT, D], bf16, tag="W_read")
    nc.gpsimd.dma_start(W_read[:], ins["W_read"][:].rearrange("(a p) w -> p a w", p=P))
    ro = ctx.enter_context(tc.tile_pool(name="ro", bufs=2))

    def readout_pre(rows_b, C):
        out = ro.tile([P, DT], f32, tag="ro_mean")
        for mt in range(DT):
            pm = ps_col(1)
            for c in range(C):
                nc.tensor.matmul(out=pm, lhsT=rows_b[:, c, mt * P:(mt + 1) * P],
                                 rhs=invN[:], start=(c == 0), stop=(c == C - 1))
            nc.vector.tensor_copy(out[:, mt:mt + 1], pm)
        return out

    def readout_post(rows_b, matT, meanT_rep, C, unscale=None):
        gt = ro.tile([P, DT], bf16, tag="ro_g")
        mb = ro.tile([P, DT], bf16, tag="ro_mb")
        nc.vector.tensor_copy(mb[:], meanT_rep)
        for mt in range(DT):
            pg = ps_col(1)
            for kt in range(DT):
                nc.tensor.matmul(out=pg,
                                 lhsT=W_read[:, kt, mt * P:(mt + 1) * P],
                                 rhs=mb[:, kt:kt + 1],
                                 start=(kt == 0), stop=(kt == DT - 1))
            gf = ro.tile([P, 1], f32, tag="ro_gf")
            nc.scalar.activation(gf[:], pg, AF.Tanh)
            nc.vector.tensor_copy(gt[:, mt:mt + 1], gf[:])
        sT = ro.tile([P, C], bf16, tag="ro_sT")
        for c in range(C):
            pss = ps_col(1)
            for kt in range(DT):
                nc.tensor.matmul(out=pss, lhsT=matT[:, kt, c * P:(c + 1) * P],
                                 rhs=gt[:, kt:kt + 1],
                                 start=(kt == 0), stop=(kt == DT - 1))
            sf = ro.tile([P, 1], f32, tag="ro_sf")
            if unscale is not None:
                nc.vector.tensor_scalar(out=sf[:], in0=pss,
                                        scalar1=unscale[:, c:c + 1],
                                        scalar2=None, op0=OP.mult)
                nc.scalar.activation(sf[:], sf[:], AF.Sigmoid)
            else:
                nc.scalar.activation(sf[:], pss, AF.Sigmoid)
            nc.vector.tensor_copy(sT[:, c:c + 1], sf[:])
        out = ro.tile([P, DT], f32, tag="ro_hg")
        for mt in range(DT):
            phh = ps_col(1)
            for c in range(C):
                nc.tensor.matmul(out=phh,
                                 lhsT=rows_b[:, c, mt * P:(mt + 1) * P],
                                 rhs=sT[:, c:c + 1], start=(c == 0),
                                 stop=(c == C - 1))
            nc.vector.tensor_copy(out[:, mt:mt + 1], phh)
        return out

    # ---- h1g/h2g ----
    mean_parts = ro.tile([P, 2 * DT], f32, tag="meanparts")
    for gi, g in enumerate("st"):
        mp = readout_pre(gs[g].h_rows_b, NT)
        nc.vector.tensor_copy(mean_parts[:, gi * DT:(gi + 1) * DT], mp[:])
    arm = allreduce(mean_parts[:], [P, 2 * DT])
    mean_rep = sb.tile([P, 2 * DT], f32, tag="meanrep")
    nc.sync.dma_start(mean_rep[:], arm[:])
    hg_parts = ro.tile([P, 2 * DT], f32, tag="hgparts")
    for gi, g in enumerate("st"):
        st = gs[g]
        hgp = readout_post(st.h_rows_b, st.hTs,
                           mean_rep[:, gi * DT:(gi + 1) * DT], NT,
                           unscale=st.nrm)
        nc.vector.tensor_copy(hg_parts[:, gi * DT:(gi + 1) * DT], hgp[:])
    arh = allreduce(hg_parts[:], [P, 2 * DT])
    hg_rep = sb.tile([P, 2 * DT], f32, tag="hgrep")
    nc.sync.dma_start(hg_rep[:], arh[:])
    for gi, g in enumerate("st"):
        gs[g].hgT = hg_rep[:, gi * DT:(gi + 1) * DT]
    if STAGE <= 4:
        zf0 = sb.tile([1, 1], f32, tag="zf0")
        nc.vector.tensor_copy(zf0[:], hg_rep[0:1, 0:1])
        nc.sync.dma_start(z_out[:], zf0[:])
        ctx.close()
        return

    if STAGE == 26:
        mph = ExitStack()
        mut = mph.enter_context(tc.tile_pool(name="mut", bufs=2))
        mut1 = mph.enter_context(tc.tile_pool(name="mut1", bufs=1))
        sta, stb = gs["s"], gs["t"]
        q = mut1.tile([P, DT, 1], bf16, tag="mq")
        qf = mut1.tile([P, DT], f32, tag="mqf")
        for ft in range(DT):
            nc.vector.reduce_sum(qf[:, ft:ft + 1], sta.hTs[:, ft, :], AX.X)
        nc.vector.tensor_copy(q[:, :, 0], qf[:])
        cosT = mut1.tile([P, JT, NL], bf16, tag="cosT")
        denp = mut1.tile([P, JT], f32, tag="denp")
        for jt in range(JT):
            pc = ps_big()
            pq = ps_col(1)
            lhs = mut.tile([P, DT, P], bf16, tag="mlhs")
            r = (jt * P) // NL
            j0 = (jt * P) % NL
            nc.sync.dma_start(
                lhs[:],
                stb.hTs_full[r, :, j0:j0 + P].rearrange("(a p) w -> p a w", p=P))
            for kt in range(DT):
                nc.tensor.matmul(out=pc[:, 0:512], lhsT=lhs[:, kt, :],
                                 rhs=sta.hTs[:, kt, 0:512],
                                 start=(kt == 0), stop=(kt == DT - 1))
                nc.tensor.matmul(out=pc[:, 512:NL], lhsT=lhs[:, kt, :],
                                 rhs=sta.hTs[:, kt, 512:NL],
                                 start=(kt == 0), stop=(kt == DT - 1))
                nc.tensor.matmul(out=pq, lhsT=lhs[:, kt, :], rhs=q[:, kt, :],
                                 start=(kt == 0), stop=(kt == DT - 1))
            nc.scalar.activation(cosT[:, jt, :], pc[:], AF.Copy)
            nc.vector.tensor_copy(denp[:, jt:jt + 1], pq)
        zf0 = sb.tile([1, 1], f32, tag="zf0")
        nc.vector.tensor_copy(zf0[:], cosT[0:1, 63, 0:1])
        nc.sync.dma_start(z_out[:], zf0[:])
        mph.close()
        ctx.close()
        return

    if STAGE == 25:
        V25 = int(os.environ.get("K_V25", "7"))
        mph = ExitStack()
        mut = mph.enter_context(tc.tile_pool(name="mut", bufs=2))
        mut1 = mph.enter_context(tc.tile_pool(name="mut1", bufs=1))
        sta, stb = gs["s"], gs["t"]
        q = mut1.tile([P, DT, 1], bf16, tag="mq")
        qf = mut1.tile([P, DT], f32, tag="mqf")
        for ft in range(DT):
            nc.vector.reduce_sum(qf[:, ft:ft + 1], sta.hTs[:, ft, :], AX.X)
        nc.vector.tensor_copy(q[:, :, 0], qf[:])
        cosT = mut1.tile([P, 1, NL], bf16, tag="cosT25")
        pc = ps_big()
        pq = ps_col(1)
        lhs = mut.tile([P, DT, P], bf16, tag="mlhs")
        nc.sync.dma_start(
            lhs[:],
            stb.hTs_full[0, :, 0:P].rearrange("(a p) w -> p a w", p=P))
        for kt in range(DT):
            if V25 & 1:
                nc.tensor.matmul(out=pc[:, 0:512], lhsT=lhs[:, kt, :],
                                 rhs=sta.hTs[:, kt, 0:512],
                                 start=(kt == 0), stop=(kt == DT - 1))
            if V25 & 2:
                nc.tensor.matmul(out=pc[:, 512:NL], lhsT=lhs[:, kt, :],
                                 rhs=sta.hTs[:, kt, 512:NL],
                                 start=(kt == 0), stop=(kt == DT - 1))
            if V25 & 4:
                nc.tensor.matmul(out=pq, lhsT=lhs[:, kt, :], rhs=q[:, kt, :],
                                 start=(kt == 0), stop=(kt == DT - 1))
        if V25 & 3:
            nc.scalar.activation(cosT[:, 0, :], pc[:], AF.Copy)
        zf0 = sb.tile([1, 1], f32, tag="zf0")
        nc.vector.tensor_copy(zf0[:], cosT[0:1, 0, 0:1])
        nc.sync.dma_start(z_out[:], zf0[:])
        mph.close()
        ctx.close()
        return

    # ================= mutual attention =================
    mph = ExitStack()
    mut = mph.enter_context(tc.tile_pool(name="mut", bufs=2))
    mut1 = mph.enter_context(tc.tile_pool(name="mut1", bufs=1))

    def mutual(ga, gb):
        sta, stb = gs[ga], gs[gb]
        q = mut1.tile([P, DT, 1], bf16, tag="mq")
        qf = mut1.tile([P, DT], f32, tag="mqf")
        for ft in range(DT):
            nc.vector.reduce_sum(qf[:, ft:ft + 1], sta.hTs[:, ft, :], AX.X)
        nc.vector.tensor_copy(q[:, :, 0], qf[:])
        cosT = mut1.tile([P, JT, NL], bf16, tag="cosT")
        denp = mut1.tile([P, JT], f32, tag="denp")
        for jt in range(JT):
            pc = ps_big()
            pq = ps_col(1)
            lhs = mut.tile([P, DT, P], bf16, tag="mlhs")
            r = (jt * P) // NL
            j0 = (jt * P) % NL
            nc.sync.dma_start(
                lhs[:],
                stb.hTs_full[r, :, j0:j0 + P].rearrange("(a p) w -> p a w", p=P))
            for kt in range(DT):
                nc.tensor.matmul(out=pc[:, 0:512], lhsT=lhs[:, kt, :],
                                 rhs=sta.hTs[:, kt, 0:512],
                                 start=(kt == 0), stop=(kt == DT - 1))
                nc.tensor.matmul(out=pc[:, 512:NL], lhsT=lhs[:, kt, :],
                                 rhs=sta.hTs[:, kt, 512:NL],
                                 start=(kt == 0), stop=(kt == DT - 1))
                nc.tensor.matmul(out=pq, lhsT=lhs[:, kt, :], rhs=q[:, kt, :],
                                 start=(kt == 0), stop=(kt == DT - 1))
            nc.scalar.activation(cosT[:, jt, :], pc[:], AF.Copy)
            nc.vector.tensor_copy(denp[:, jt:jt + 1], pq)
        ard = allreduce(denp[:], [P, JT])
        den = mut1.tile([P, JT], f32, tag="den")
        nc.sync.dma_start(den[:], ard[:])
        rden = mut1.tile([P, JT], f32, tag="rden")
        nc.vector.reciprocal(rden[:], den[:])
        hm = mut1.tile([P, NT, D], f32, tag=f"hm_{ga}")
        for it in range(NT):
            pm = ps_main(D)
            for jt in range(JT):
                h2s = mut.tile([P, D], bf16, tag="h2s")
                nc.sync.dma_start(h2s[:], stb.hR_full[jt * P:(jt + 1) * P, :])
                nc.vector.tensor_scalar(out=h2s[:], in0=h2s[:],
                                        scalar1=rden[:, jt:jt + 1],
                                        scalar2=None, op0=OP.mult)
                nc.tensor.matmul(out=pm,
                                 lhsT=cosT[:, jt, it * P:(it + 1) * P],
                                 rhs=h2s[:], start=(jt == 0),
                                 stop=(jt == JT - 1))
            nc.vector.tensor_copy(hm[:, it, :], pm)
        return hm

    if STAGE == 27:
        hm27 = mutual("s", "t")
        zf0 = sb.tile([1, 1], f32, tag="zf0")
        nc.vector.tensor_copy(zf0[:], hm27[0:1, 0, 0:1])
        nc.sync.dma_start(z_out[:], zf0[:])
        mph.close()
        ctx.close()
        return
    NOTR = os.environ.get("K_NOTR", "") == "1"
    for g, o in (("s", "t"), ("t", "s")):
        st = gs[g]
        st.hm = mutual(g, o)
        st.hm_b = sb.tile([P, NT, D], bf16, tag=f"hmb_{g}")
        nc.vector.tensor_copy(st.hm_b[:], st.hm[:])
        st.hmT = sb.tile([P, DT, NL], bf16, tag=f"hmT_{g}")
        if NOTR:
            nc.gpsimd.memset(st.hmT[:], 0.001)
        else:
            for t in range(NT):
                for ft in range(DT):
                    pe_transpose(st.hm_b[:, t, ft * P:(ft + 1) * P],
                                 st.hmT[:, ft, t * P:(t + 1) * P])
    mph.close()
    if STAGE <= 5:
        zf0 = sb.tile([1, 1], f32, tag="zf0")
        nc.vector.tensor_copy(zf0[:], gs["s"].hm_b[0:1, 0, 0:1])
        nc.sync.dma_start(z_out[:], zf0[:])
        ctx.close()
        return

    # hm readouts
    m2_parts = ro.tile([P, 2 * DT], f32, tag="m2parts")
    for gi, g in enumerate("st"):
        st = gs[g]
        mp = readout_pre(st.hm_b, NT)
        nc.vector.tensor_copy(m2_parts[:, gi * DT:(gi + 1) * DT], mp[:])
    arm2 = allreduce(m2_parts[:], [P, 2 * DT])
    m2_rep = sb.tile([P, 2 * DT], f32, tag="m2rep")
    nc.sync.dma_start(m2_rep[:], arm2[:])
    hmg_parts = ro.tile([P, 2 * DT], f32, tag="hmgparts")
    for gi, g in enumerate("st"):
        st = gs[g]
        hgp = readout_post(st.hm_b, st.hmT,
                           m2_rep[:, gi * DT:(gi + 1) * DT], NT)
        nc.vector.tensor_copy(hmg_parts[:, gi * DT:(gi + 1) * DT], hgp[:])
    arh2 = allreduce(hmg_parts[:], [P, 2 * DT])
    hmg_rep = sb.tile([P, 2 * DT], f32, tag="hmgrep")
    nc.sync.dma_start(hmg_rep[:], arh2[:])
    for gi, g in enumerate("st"):
        gs[g].hmgT = hmg_rep[:, gi * DT:(gi + 1) * DT]

    # ================= matches =================
    ma = ctx.enter_context(tc.tile_pool(name="match", bufs=1))
    W2mb = sb.tile([P, DT, MS], bf16, tag="W2mb")
    nc.gpsimd.dma_start(W2mb[:], ins["W2m"][:].rearrange("(a p) w -> p a w", p=P))

    def mm_w2m(prodT, X, tag):
        out = ma.tile([P, MT, X], f32, tag=tag)
        for mt in range(MT):
            for x0 in range(0, X, 512):
                xw = min(512, X - x0)
                pm = ps_main(xw)
                for kt in range(DT):
                    nc.tensor.matmul(out=pm,
                                     lhsT=W2mb[:, kt, mt * P:(mt + 1) * P],
                                     rhs=prodT[:, kt, x0:x0 + xw],
                                     start=(kt == 0), stop=(kt == DT - 1))
                nc.vector.tensor_copy(out[:, mt, x0:x0 + xw], pm)
        return out

    def match_full(dotT, n1T, n2T, X):
        t = ma.tile([P, MT, X], f32, tag="mt_tmp")
        nc.vector.tensor_tensor(out=t[:], in0=n1T[:], in1=n2T[:], op=OP.mult)
        nc.vector.tensor_scalar(out=t[:], in0=t[:], scalar1=1e-16,
                                scalar2=None, op0=OP.max)
        nc.scalar.activation(t[:], t[:], AF.Abs_reciprocal_sqrt)
        nc.vector.tensor_tensor(out=dotT[:], in0=dotT[:], in1=t[:], op=OP.mult)
        return dotT

    def match_col(dotT, nT, ncol, X):
        """nT [P,MT,X], ncol [P,MT,1]; dotT scaled in place."""
        t = ma.tile([P, MT, X], f32, tag="mt_tmp")
        for mt in range(MT):
            nc.vector.tensor_scalar(out=t[:, mt, :], in0=nT[:, mt, :],
                                    scalar1=ncol[:, mt, :], scalar2=None,
                                    op0=OP.mult)
        nc.vector.tensor_scalar(out=t[:], in0=t[:], scalar1=1e-16,
                                scalar2=None, op0=OP.max)
        nc.scalar.activation(t[:], t[:], AF.Abs_reciprocal_sqrt)
        nc.vector.tensor_tensor(out=dotT[:], in0=dotT[:], in1=t[:], op=OP.mult)
        return dotT

    def transpose_to_rows(matT_f32, tag):
        mb = ma.tile([P, MT, NL], bf16, tag=tag + "_Tb")
        nc.vector.tensor_copy(mb[:], matT_f32[:])
        rows = ma.tile([P, NT, MS], bf16, tag=tag + "_rows")
        for mt in range(MT):
            for t in range(NT):
                pe_transpose(mb[:, mt, t * P:(t + 1) * P],
                             rows[:, t, mt * P:(mt + 1) * P])
        return mb, rows

    def colsq_col(col_f32, tag):
        """[P, DT] f32 -> squared bf16 [P, DT, 1]"""
        o = ma.tile([P, DT, 1], bf16, tag=tag)
        t = ma.tile([P, DT], f32, tag=tag + "_f")
        nc.vector.tensor_tensor(out=t[:], in0=col_f32, in1=col_f32, op=OP.mult)
        nc.vector.tensor_copy(o[:, :, 0], t[:])
        return o

    match_rows = {}
    match_Tb = {}
    for gi, g in enumerate("st"):
        st = gs[g]
        hTq = ma.tile([P, DT, NL], bf16, tag="hTq")
        nc.vector.tensor_tensor(out=hTq[:], in0=st.hT[:], in1=st.hT[:],
                                op=OP.mult)
        nsq_h = mm_w2m(hTq, NL, "nsq_h")
        nc.vector.tensor_tensor(out=hTq[:], in0=st.hmT[:], in1=st.hmT[:],
                                op=OP.mult)
        nsq_m = mm_w2m(hTq, NL, "nsq_m")
        # miu
        nc.vector.tensor_tensor(out=hTq[:], in0=st.hT[:], in1=st.hmT[:],
                                op=OP.mult)
        dot = mm_w2m(hTq, NL, "dot_miu")
        miuT = match_full(dot, nsq_h, nsq_m, NL)
        match_Tb[f"miu{gi}"], match_rows[f"miu{gi}"] = \
            transpose_to_rows(miuT, f"miu{gi}")
        # phi: h x hmg_bcast
        for ft in range(DT):
            nc.vector.tensor_scalar(out=hTq[:, ft, :], in0=st.hT[:, ft, :],
                                    scalar1=st.hmgT[:, ft:ft + 1], scalar2=None,
                                    op0=OP.mult)
        dotp = mm_w2m(hTq, NL, "dot_phi")
        n2c = mm_w2m(colsq_col(st.hmgT, "hmgsq"), 1, "n2c_phi")
        phiT = match_col(dotp, nsq_h, n2c, NL)
        match_Tb[f"phi{gi}"], match_rows[f"phi{gi}"] = \
            transpose_to_rows(phiT, f"phi{gi}")
        # psi: hg_bcast x hm
        for ft in range(DT):
            nc.vector.tensor_scalar(out=hTq[:, ft, :], in0=st.hmT[:, ft, :],
                                    scalar1=st.hgT[:, ft:ft + 1], scalar2=None,
                                    op0=OP.mult)
        dots = mm_w2m(hTq, NL, "dot_psi")
        n1c = mm_w2m(colsq_col(st.hgT, "hgsq"), 1, "n1c_psi")
        psiT = match_col(dots, nsq_m, n1c, NL)
        match_Tb[f"psi{gi}"], match_rows[f"psi{gi}"] = \
            transpose_to_rows(psiT, f"psi{gi}")
        # om: hg x hmg (columns only)
        po = ma.tile([P, DT, 1], bf16, tag="po")
        pof = ma.tile([P, DT], f32, tag="pof")
        nc.vector.tensor_tensor(out=pof[:], in0=st.hgT, in1=st.hmgT, op=OP.mult)
        nc.vector.tensor_copy(po[:, :, 0], pof[:])
        dot_om = mm_w2m(po, 1, "dot_om")
        n1o = mm_w2m(colsq_col(st.hgT, "hgsq"), 1, "n1o")
        n2o = mm_w2m(colsq_col(st.hmgT, "hmgsq"), 1, "n2o")
        tcol = ma.tile([P, MT, 1], f32, tag="omt")
        nc.vector.tensor_tensor(out=tcol[:], in0=n1o[:], in1=n2o[:], op=OP.mult)
        nc.vector.tensor_scalar(out=tcol[:], in0=tcol[:], scalar1=1e-16,
                                scalar2=None, op0=OP.max)
        nc.scalar.activation(tcol[:], tcol[:], AF.Abs_reciprocal_sqrt)
        nc.vector.tensor_tensor(out=dot_om[:], in0=dot_om[:], in1=tcol[:],
                                op=OP.mult)
        st.omT = sb.tile([P, MT], f32, tag=f"om_{g}")
        nc.vector.tensor_copy(st.omT[:], dot_om[:, :, 0])

    # match readouts: means (one AR), then posts (one AR)
    mm_parts = ro.tile([P, 6 * MT], f32, tag="mmparts")
    names = ["miu0", "phi0", "psi0", "miu1", "phi1", "psi1"]
    for i, nm in enumerate(names):
        mp = readout_pre(match_rows[nm], NT)
        nc.vector.tensor_copy(mm_parts[:, i * MT:(i + 1) * MT], mp[:])
    arm3 = allreduce(mm_parts[:], [P, 6 * MT])
    mm_rep = ro.tile([P, 6 * MT], f32, tag="mmrep")
    nc.sync.dma_start(mm_rep[:], arm3[:])
    mg_parts = ro.tile([P, 6 * MT], f32, tag="mgparts")
    for i, nm in enumerate(names):
        hgp = readout_post(match_rows[nm], match_Tb[nm],
                           mm_rep[:, i * MT:(i + 1) * MT], NT)
        nc.vector.tensor_copy(mg_parts[:, i * MT:(i + 1) * MT], hgp[:])
    arh3 = allreduce(mg_parts[:], [P, 6 * MT])
    mg_rep = ro.tile([P, 6 * MT], f32, tag="mgrep")
    nc.sync.dma_start(mg_rep[:], arh3[:])

    # ================= z + MLP =================
    zT = ro.tile([P, 16], f32, tag="zT")
    # order: miu1g phi1g psi1g om1 | miu2g phi2g psi2g om2 (each 2 cols)
    for gi in range(2):
        o = 8 * gi
        nc.vector.tensor_copy(zT[:, o + 0:o + 2],
                              mg_rep[:, (3 * gi + 0) * MT:(3 * gi + 1) * MT])
        nc.vector.tensor_copy(zT[:, o + 2:o + 4],
                              mg_rep[:, (3 * gi + 1) * MT:(3 * gi + 2) * MT])
        nc.vector.tensor_copy(zT[:, o + 4:o + 6],
                              mg_rep[:, (3 * gi + 2) * MT:(3 * gi + 3) * MT])
        nc.vector.tensor_copy(zT[:, o + 6:o + 8],
                              gs["st"[gi]].omT[:])
    zb = ro.tile([P, 16], bf16, tag="zb")
    nc.vector.tensor_copy(zb[:], zT[:])

    mlp_dims = [(16, 4, "mlp_w1", "mlp_b1"), (4, 2, "mlp_w2", "mlp_b2"),
                (2, 1, "mlp_w3", "mlp_b3")]
    mlpp = ctx.enter_context(tc.tile_pool(name="mlpp", bufs=1))
    cur = zb
    for (KT2, MT2, wn, bn) in mlp_dims:
        wb = mlpp.tile([P, KT2, MT2 * P], bf16, tag=wn + "_b")
        nc.gpsimd.dma_start(wb[:], ins[wn][:].rearrange("(a p) w -> p a w", p=P))
        bias = mlpp.tile([P, MT2], f32, tag=bn)
        nc.sync.dma_start(bias[:], ins[bn][:])
        nxt = mlpp.tile([P, MT2], bf16, tag=wn + "_out")
        for mt in range(MT2):
            pm = ps_col(1)
            for kt in range(KT2):
                nc.tensor.matmul(out=pm,
                                 lhsT=wb[:, kt, mt * P:(mt + 1) * P],
                                 rhs=cur[:, kt:kt + 1],
                                 start=(kt == 0), stop=(kt == KT2 - 1))
            of = mlpp.tile([P, 1], f32, tag=wn + "_of")
            nc.scalar.activation(of[:], pm, AF.Relu, bias=bias[:, mt:mt + 1])
            nc.vector.tensor_copy(nxt[:, mt:mt + 1], of[:])
        cur = nxt
    # final: w4 [128, 1]
    w4b = mlpp.tile([P, 1], bf16, tag="w4b")
    nc.gpsimd.dma_start(w4b[:], ins["mlp_w4"][:])
    b4 = mlpp.tile([1, 1], f32, tag="b4")
    nc.sync.dma_start(b4[:], ins["mlp_b4"][:])
    pz = ps_col(1)[0:1, :]
    nc.tensor.matmul(out=pz, lhsT=w4b[:], rhs=cur[:, 0:1], start=True,
                     stop=True)
    zf = ro.tile([1, 1], f32, tag="zf")
    nc.scalar.activation(zf[:], pz, AF.Sigmoid, bias=b4[:])
    nc.sync.dma_start(z_out[:], zf[:])
    ctx.close()


# ------------------------------------------------------------------ entry
def kernel(**inputs):
    in_maps, EB = _host_prep(inputs)
    key = EB
    if key not in _CACHE:
        _CACHE[key] = build(EB)
    nc = _CACHE[key]
    res = run_bass_kernel_spmd(nc, in_maps, list(range(NC)))
    z = np.asarray(res.results[0]["z"], dtype=np.float32)
    label = np.asarray(inputs["label"])
    label_exp = np.exp(-label.astype(np.float32))
    return z, label, label_exp
